# revision 17
# baseline (speedup 1.0000x reference)
"""Llama GQA attention (B=2, S=2048, H=4096, 32 q heads / 8 kv heads, HD=128)
on 8 Trainium2 NeuronCores.

Sharding: TP=8 over heads; every core processes BOTH batches.
  core r: q heads [4r, 4r+4), kv head r, out cols [512r, 512(r+1)).
  AllGather (bf16) of attention outputs over all 8 cores per token
  chunk; each core then projects the full 4096 attn features to its
  512 output columns -> disjoint outputs, host concatenates.

All inputs are cast to bf16 host-side (weights pre-rearranged into
[128, kt*128] lhsT panels), so the kernel does no casting and reads
each weight exactly once into SBUF where it stays resident (~10MB).

On-chip layout is fully "transposed" ([feature, token]):
  QT/KT: [d, t] (weight panels stationary, X^T moving)
  V^T:   [d, t] computed like K, then PE is_transpose matmuls
         (vs identity) build V[t, d] for the PV matmul lhsT.
  S^T[k, q] = (KT tile).T @ QT          (contraction d on partitions)
  P^T = exp(scale * S^T)                (ScalarE, fp32 PSUM -> bf16 SBUF)
  attn^T[d, q] += (V tile).T @ P^T      (contraction k-tokens on partitions)
  denom[*, q] += ones128.T @ P^T        (col-sums replicated on partitions)
  out^T[oc, t] += (wo tile).T @ af^T    (contraction features on partitions)
Causal masking: only lower-triangle k-tiles are computed; diagonal
128-tile blocks use static 0/1 masks (DVE multiply into P^T).  Softmax
skips max-subtraction (scores are O(8), exp fits fp32 comfortably).
1/denominator uses the fast custom-DVE reciprocal (~5x vs InstReciprocal).

Projections are kt-outer (stream X^T tiles, accumulate 4xQ + K + V^T in
six PSUM banks).  PSUM fits in 8 banks via three shared tag rings:
  quad(4): pq0..3 | sps     duo(2): pk, pvt | pa     uno(2): pd | po
The chunk is the proj/rope granularity (512 tokens); attention + AG +
outproj run per "job" (the last chunk is split 256/128/128 so the
final AllGather + output-projection tail shrinks toward the AG floor).

Engine rings: Sync HWDGE = X^T tiles + weights (latency-critical,
clean DMAHW lanes); Scalar HWDGE = rope swap DMAs; GpSimd SWDGE =
collectives + the collective-adjacent bulk DMAs (cci/af/out), whose
AllGather-gated pending-ness must stay off the shared DMAHW lane
semaphores (it poisons every later HWDGE consumer on the same lane)
and off the compute engines.  The output projection lags its
AllGather by TWO jobs so af transfers always have a window of slack.
"""

import sys

for _p in ("/opt/trn_rl_repo",):
    if _p not in sys.path:
        sys.path.append(_p)

import numpy as np
import ml_dtypes

import concourse.bacc as bacc
import concourse.mybir as mybir
import concourse.tile as tile
from concourse.bass_utils import run_bass_kernel_spmd

F32 = mybir.dt.float32
BF16 = mybir.dt.bfloat16
BF = ml_dtypes.bfloat16

B, S, H = 2, 2048, 4096
NH, NKV, HD = 32, 8, 128
N_CORES = 8
GROUPS = [[0, 1, 2, 3, 4, 5, 6, 7]]

HL = 4                 # local q heads
OC = H // N_CORES      # 512 local out cols
TC = 512               # token chunk (proj granularity)
NCB = S // TC          # 4 chunks per batch
KT = H // 128          # 32 contraction tiles for the projections
SCALE = float(HD ** -0.5)

LAST_RESULT = None
_BUILT = {}


def _build():
    nc = bacc.Bacc("TRN2", debug=False, num_devices=N_CORES)

    xt_d = nc.dram_tensor("xt", [H, B * S], BF16, kind="ExternalInput").ap()
    cos_d = nc.dram_tensor("cos_t", [HD, B * S], F32, kind="ExternalInput").ap()
    sin_d = nc.dram_tensor("sin_t", [HD, B * S], F32, kind="ExternalInput").ap()
    wq_d = nc.dram_tensor("wq_s", [128, HL * KT * 128], BF16,
                          kind="ExternalInput").ap()
    wk_d = nc.dram_tensor("wk_s", [128, KT * 128], BF16,
                          kind="ExternalInput").ap()
    wv_d = nc.dram_tensor("wv_s", [128, KT * 128], BF16,
                          kind="ExternalInput").ap()
    wo_d = nc.dram_tensor("wo_s", [128, 4 * KT * 128], BF16,
                          kind="ExternalInput").ap()
    mask_d = nc.dram_tensor("masks", [HD, 4 * TC], BF16,
                            kind="ExternalInput").ap()
    ones_d = nc.dram_tensor("onesb", [128, 128], BF16,
                            kind="ExternalInput").ap()
    iden_d = nc.dram_tensor("ident", [128, 128], BF16,
                            kind="ExternalInput").ap()
    out_d = nc.dram_tensor("out_t", [OC, B * S], F32, kind="ExternalOutput").ap()

    with tile.TileContext(nc) as tc:
        with tc.tile_pool(name="sb", bufs=1) as sb, \
             tc.tile_pool(name="ps", bufs=1, space="PSUM") as ps, \
             tc.tile_pool(name="dr", bufs=1, space="DRAM") as dr:

            # ---- persistent tiles ----
            wqp = sb.tile([128, HL * KT * 128], BF16)   # 4MB
            wkp = sb.tile([128, KT * 128], BF16)        # 1MB
            wvp = sb.tile([128, KT * 128], BF16)        # 1MB
            wop = sb.tile([128, 4 * KT * 128], BF16)    # 4MB
            cos_sb = sb.tile([HD, B * S], F32)          # 2MB
            sin_sb = sb.tile([HD, B * S], F32)          # 2MB
            mask_sb = sb.tile([HD, 4 * TC], BF16)
            ones_sb = sb.tile([128, 128], BF16)
            iden_sb = sb.tile([128, 128], BF16)
            ktb = sb.tile([128, B * S], BF16)           # roped K^T, 1MB
            vb = sb.tile([128, (B * S // 128) * 128], BF16)  # V [t, d], 1MB

            QKT = KT * 128 // 4  # quarter-panel cols (1024)

            def load_qkv_quarter(q):
                for h in range(HL):
                    o = h * KT * 128 + q * QKT
                    nc.sync.dma_start(wqp[:, o:o + QKT], wq_d[:, o:o + QKT])
                nc.sync.dma_start(wkp[:, q * QKT:(q + 1) * QKT],
                                  wk_d[:, q * QKT:(q + 1) * QKT])
                nc.sync.dma_start(wvp[:, q * QKT:(q + 1) * QKT],
                                  wv_d[:, q * QKT:(q + 1) * QKT])

            load_qkv_quarter(0)

            def late_loads(kt):
                """remaining startup DMAs, interleaved into chunk-0 X^T DMAs."""
                if kt == 1:
                    load_qkv_quarter(1)
                elif kt == 3:
                    nc.sync.dma_start(cos_sb[:], cos_d[:])
                elif kt == 5:
                    nc.sync.dma_start(sin_sb[:], sin_d[:])
                elif kt == 7:
                    nc.sync.dma_start(mask_sb[:], mask_d[:])
                    nc.sync.dma_start(ones_sb[:], ones_d[:])
                    nc.sync.dma_start(iden_sb[:], iden_d[:])
                elif kt == 9:
                    load_qkv_quarter(2)
                elif kt == 13:
                    load_qkv_quarter(3)
                elif kt in (18, 22, 26, 30):
                    i = (kt - 18) // 4  # wo quarter
                    nc.sync.dma_start(wop[:, i * QKT * 4:(i + 1) * QKT * 4],
                                      wo_d[:, i * QKT * 4:(i + 1) * QKT * 4])

            def rope(dst, pq, t0g):
                """dst (bf16 [128, TC]) = rope of pq (fp32 PSUM [128, TC]).
                Swap DMAs ride the Scalar HWDGE ring, right behind the ACT
                copy that produces their input."""
                qf = sb.tile([128, TC], F32, tag="qf", bufs=2, name="qf")
                nc.scalar.copy(qf[:], pq[:])
                qs = sb.tile([128, TC], F32, tag="qs", bufs=2, name="qs")
                nc.scalar.dma_start(qs[0:64, :], qf[64:128, :])
                nc.scalar.dma_start(qs[64:128, :], qf[0:64, :])
                nc.vector.tensor_tensor(
                    qf[:], qf[:], cos_sb[:, t0g:t0g + TC], mybir.AluOpType.mult)
                nc.vector.tensor_tensor(
                    qs[:], qs[:], sin_sb[:, t0g:t0g + TC], mybir.AluOpType.mult)
                nc.vector.tensor_tensor(dst, qf[:], qs[:], mybir.AluOpType.add)

            def proj(c):
                """kt-outer Q/K/V^T projections + rope for chunk c."""
                b, ci = c // NCB, c % NCB
                t0g = b * S + ci * TC
                pq = [ps.tile([128, TC], F32, tag="quad", bufs=4,
                              name=f"pq{h}") for h in range(HL)]
                pk = ps.tile([128, TC], F32, tag="duo", bufs=2, name="pk")
                pvt = ps.tile([128, TC], F32, tag="duo", bufs=2, name="pvt")
                for kt in range(KT):
                    xt = sb.tile([128, TC], BF16, tag="xt", bufs=10, name="xt")
                    nc.sync.dma_start(
                        xt[:], xt_d[kt * 128:(kt + 1) * 128, t0g:t0g + TC])
                    if c == 0:
                        late_loads(kt)
                    st, sp = (kt == 0), (kt == KT - 1)
                    for h in range(HL):
                        nc.tensor.matmul(
                            pq[h][:],
                            wqp[:, h * KT * 128 + kt * 128:
                                h * KT * 128 + (kt + 1) * 128],
                            xt[:], start=st, stop=sp)
                    nc.tensor.matmul(
                        pk[:], wkp[:, kt * 128:(kt + 1) * 128], xt[:],
                        start=st, stop=sp)
                    nc.tensor.matmul(
                        pvt[:], wvp[:, kt * 128:(kt + 1) * 128], xt[:],
                        start=st, stop=sp)
                # K first (every S^T of this chunk needs it), then V
                # (PE transpose via identity), then Q heads.
                rope(ktb[:, t0g:t0g + TC], pk, t0g)
                vts = sb.tile([128, TC], BF16, tag="vts", bufs=1, name="vts")
                nc.scalar.copy(vts[:], pvt[:])
                ptr = ps.tile([128, TC], BF16, tag="uno", bufs=2, name="ptr")
                for tb in range(TC // 128):
                    nc.tensor.matmul(
                        ptr[:, tb * 128:(tb + 1) * 128],
                        vts[:, tb * 128:(tb + 1) * 128], iden_sb[:],
                        is_transpose=True,
                        start=(tb == 0), stop=(tb == TC // 128 - 1))
                blk0 = t0g // 128
                nc.vector.tensor_copy(
                    vb[:, blk0 * 128:blk0 * 128 + TC], ptr[:])
                qtb = sb.tile([128, HL * TC], BF16, tag="qt", bufs=2,
                              name="qtb")
                for h in range(HL):
                    rope(qtb[:, h * TC:(h + 1) * TC], pq[h], t0g)
                return qtb

            def attn_head(job, h, qtb, attnb):
                c, q0l, tn = job
                b, ci = c // NCB, c % NCB
                kb0 = b * S // 128                      # batch base k-tile
                nkt = (ci * TC + q0l + tn) // 128
                dg0 = (ci * TC + q0l) // 128            # first masked k-tile
                qsl = qtb[:, h * TC + q0l:h * TC + q0l + tn]
                pa = ps.tile([128, tn], F32, tag="duo", bufs=2, name="pa")
                pd = ps.tile([128, tn], F32, tag="uno", bufs=2, name="pd")
                for kt in range(nkt):
                    sps = ps.tile([128, tn], F32, tag="quad", bufs=4,
                                  name="sps")
                    nc.tensor.matmul(
                        sps[:],
                        ktb[:, (kb0 + kt) * 128:(kb0 + kt + 1) * 128],
                        qsl, start=True, stop=True)
                    pt = sb.tile([128, tn], BF16, tag="pt", bufs=3, name="pt")
                    nc.scalar.activation(
                        pt[:], sps[:], mybir.ActivationFunctionType.Exp,
                        scale=SCALE)
                    if kt >= dg0:
                        j = kt - 4 * ci
                        nc.vector.tensor_tensor(
                            pt[:], pt[:],
                            mask_sb[:, j * TC + q0l:j * TC + q0l + tn],
                            mybir.AluOpType.mult)
                    nc.tensor.matmul(
                        pa[:], vb[:, (kb0 + kt) * 128:(kb0 + kt + 1) * 128],
                        pt[:], start=(kt == 0), stop=(kt == nkt - 1))
                    nc.tensor.matmul(
                        pd[:], ones_sb[:], pt[:],
                        start=(kt == 0), stop=(kt == nkt - 1))
                rc = sb.tile([128, tn], F32, tag="rc", bufs=2, name="rc")
                nc.vector.reciprocal_approx_fast(rc[:], pd[:])
                nc.vector.tensor_tensor(
                    attnb[:, h * tn:(h + 1) * tn], pa[:], rc[:],
                    mybir.AluOpType.mult)

            def outproj_block(job, oc, af):
                c, q0l, tn = job
                b, ci = c // NCB, c % NCB
                t0 = b * S + ci * TC + q0l
                po = ps.tile([128, tn], F32, tag="uno", bufs=2, name="po")
                for dt in range(KT):
                    nc.tensor.matmul(
                        po[:],
                        wop[:, oc * KT * 128 + dt * 128:
                            oc * KT * 128 + (dt + 1) * 128],
                        af[:, dt * tn:(dt + 1) * tn],
                        start=(dt == 0), stop=(dt == KT - 1))
                ot = sb.tile([128, tn], F32, tag="ot", bufs=2, name="ot")
                nc.scalar.copy(ot[:], po[:])
                nc.gpsimd.dma_start(
                    out_d[oc * 128:(oc + 1) * 128, t0:t0 + tn], ot[:])

            def allgather(job, attnb):
                c, q0l, tn = job
                cci = dr.tile([HL * 128, tn], BF16, tag="cci", bufs=2,
                              name="cci")
                cco = dr.tile([H, tn], BF16, tag="cco", bufs=2, name="cco")
                nc.gpsimd.dma_start(
                    cci.rearrange("(h p) t -> p h t", p=128),
                    attnb.rearrange("p (h t) -> p h t", h=HL))
                nc.gpsimd.collective_compute(
                    "AllGather", mybir.AluOpType.bypass,
                    replica_groups=GROUPS, ins=[cci[:]], outs=[cco[:]])
                return cco

            def load_af(pending):
                pjob, pcco = pending
                tn = pjob[2]
                af = sb.tile([128, KT * tn], BF16, tag="af", bufs=1,
                             name="af")
                nc.gpsimd.dma_start(
                    af.rearrange("p (d t) -> p d t", d=KT),
                    pcco.rearrange("(d p) t -> p d t", p=128))
                return af

            # jobs: full 512-token chunks 0..5, then chunks 6 and 7
            # split and interleaved so every late window still has
            # attention+outproj work to hide the ~30us AllGathers, and
            # the final AllGather tail shrinks toward the AG floor
            jobs = [(c, 0, TC) for c in range(6)]
            jobs += [(6, 0, 256), (7, 0, 256), (6, 256, 256),
                     (7, 256, 128), (7, 384, 128)]

            pendings = []  # (job, cco) awaiting output projection
            qtbs = {0: proj(0)}
            for job in jobs:
                c, q0l, tn = job
                if q0l == 0 and c + 1 < B * NCB:
                    qtbs[c + 1] = proj(c + 1)
                qtb = qtbs[c]
                # outproj lags two jobs behind its AllGather, so the af
                # transfer has a full window of slack.
                pending = pendings.pop(0) if len(pendings) >= 2 else None
                af = load_af(pending) if pending is not None else None
                attnb = sb.tile([128, HL * tn], BF16, tag="attn", bufs=1,
                                name="attnb")
                for h in range(HL):
                    attn_head(job, h, qtb, attnb)
                    if pending is not None and h >= 1:
                        outproj_block(pending[0], h - 1, af)
                if pending is not None:
                    outproj_block(pending[0], 3, af)
                cco = allgather(job, attnb)
                pendings.append((job, cco))
            # drain the remaining output projections
            for pending in pendings:
                af = load_af(pending)
                for oc in range(4):
                    outproj_block(pending[0], oc, af)

    nc.compile()
    return nc


def _get_nc():
    if "nc" not in _BUILT:
        _BUILT["nc"] = _build()
    return _BUILT["nc"]


def _panelize(w):
    """[4096, 128] fp32 -> [128, 32*128] bf16 lhsT/rhs panel."""
    return np.ascontiguousarray(
        w.reshape(KT, 128, 128).transpose(1, 0, 2).reshape(128, KT * 128)
    ).astype(BF)


def kernel(hidden_states, cos, sin, wq, wk, wv, wo):
    global LAST_RESULT
    nc = _get_nc()

    hidden_states = np.asarray(hidden_states, dtype=np.float32)
    cos = np.asarray(cos, dtype=np.float32)
    sin = np.asarray(sin, dtype=np.float32)
    wq = np.asarray(wq, dtype=np.float32)
    wk = np.asarray(wk, dtype=np.float32)
    wv = np.asarray(wv, dtype=np.float32)
    wo = np.asarray(wo, dtype=np.float32)

    # host-side shard prep (bf16 casts + panel layouts)
    xt = np.concatenate([hidden_states[0].T, hidden_states[1].T],
                        axis=1).astype(BF)                    # [H, B*S]
    cos_t = np.concatenate([cos[0].T, cos[1].T], axis=1)      # [HD, B*S]
    cos_t = np.ascontiguousarray(cos_t, dtype=np.float32)
    sin_t = np.concatenate([sin[0].T, sin[1].T], axis=1).copy()
    sin_t = np.ascontiguousarray(sin_t, dtype=np.float32)
    sin_t[0:64, :] *= -1.0                                    # rotate_half sign

    masks = np.zeros((HD, 4 * TC), dtype=BF)
    for j in range(4):
        m = (np.arange(HD)[:, None] + 128 * j) <= np.arange(TC)[None, :]
        masks[:, j * TC:(j + 1) * TC] = m.astype(BF)
    onesb = np.ones((128, 128), dtype=BF)
    ident = np.eye(128, dtype=np.float32).astype(BF)

    in_maps = []
    for r in range(N_CORES):
        wq_p = np.concatenate(
            [_panelize(wq[:, (4 * r + h) * 128:(4 * r + h + 1) * 128])
             for h in range(HL)], axis=1)
        wo_p = np.concatenate(
            [_panelize(wo[:, r * OC + oc * 128:r * OC + (oc + 1) * 128])
             for oc in range(4)], axis=1)
        in_maps.append({
            "xt": xt,
            "cos_t": cos_t,
            "sin_t": sin_t,
            "wq_s": wq_p,
            "wk_s": _panelize(wk[:, r * 128:(r + 1) * 128]),
            "wv_s": _panelize(wv[:, r * 128:(r + 1) * 128]),
            "wo_s": wo_p,
            "masks": masks,
            "onesb": onesb,
            "ident": ident,
        })

    res = run_bass_kernel_spmd(nc, in_maps, core_ids=list(range(N_CORES)))
    LAST_RESULT = res

    out = np.empty((B, S, H), dtype=np.float32)
    for r in range(N_CORES):
        o = res.results[r]["out_t"]  # [OC, B*S]
        for b in range(B):
            out[b, :, r * OC:(r + 1) * OC] = o[:, b * S:(b + 1) * S].T
    return out


# revision 19
# speedup vs baseline: 1.0305x; 1.0305x over previous
"""Llama GQA attention (B=2, S=2048, H=4096, 32 q heads / 8 kv heads, HD=128)
on 8 Trainium2 NeuronCores.

Sharding: TP=8 over heads; every core processes BOTH batches.
  core r: q heads [4r, 4r+4), kv head r, out cols [512r, 512(r+1)).
  AllGather (bf16) of attention outputs over all 8 cores per token
  chunk; each core then projects the full 4096 attn features to its
  512 output columns -> disjoint outputs, host concatenates.

All inputs are cast to bf16 host-side (weights pre-rearranged into
[128, kt*128] lhsT panels), so the kernel does no casting and reads
each weight exactly once into SBUF where it stays resident (~10MB).

On-chip layout is fully "transposed" ([feature, token]):
  QT/KT: [d, t] (weight panels stationary, X^T moving)
  V^T:   [d, t] computed like K, then PE is_transpose matmuls
         (vs identity) build V[t, d] for the PV matmul lhsT.
  S^T[k, q] = (KT tile).T @ QT          (contraction d on partitions)
  P^T = exp(scale * S^T)                (ScalarE, fp32 PSUM -> bf16 SBUF)
  attn^T[d, q] += (V tile).T @ P^T      (contraction k-tokens on partitions)
  denom[*, q] += ones128.T @ P^T        (col-sums replicated on partitions)
  out^T[oc, t] += (wo tile).T @ af^T    (contraction features on partitions)
Causal masking: only lower-triangle k-tiles are computed; diagonal
128-tile blocks use static 0/1 masks (DVE multiply into P^T).  Softmax
skips max-subtraction (scores are O(8), exp fits fp32 comfortably).
1/denominator uses the fast custom-DVE reciprocal (~5x vs InstReciprocal).

Projections are kt-outer (stream X^T tiles, accumulate 4xQ + K + V^T in
six PSUM banks).  PSUM fits in 8 banks via three shared tag rings:
  quad(4): pq0..3 | sps     duo(2): pk, pvt | pa     uno(2): pd | po
The chunk is the proj/rope granularity (512 tokens); attention + AG +
outproj run per "job" (the last chunk is split 256/128/128 so the
final AllGather + output-projection tail shrinks toward the AG floor).

Engine rings: Sync HWDGE = X^T tiles + weights (latency-critical,
clean DMAHW lanes); Scalar HWDGE = rope swap DMAs; GpSimd SWDGE =
collectives + the collective-adjacent bulk DMAs (cci/af/out), whose
AllGather-gated pending-ness must stay off the shared DMAHW lane
semaphores (it poisons every later HWDGE consumer on the same lane)
and off the compute engines.  The output projection lags its
AllGather by TWO jobs so af transfers always have a window of slack.
"""

import sys

for _p in ("/opt/trn_rl_repo",):
    if _p not in sys.path:
        sys.path.append(_p)

import numpy as np
import ml_dtypes

import concourse.bacc as bacc
import concourse.mybir as mybir
import concourse.tile as tile
from concourse.bass_utils import run_bass_kernel_spmd

F32 = mybir.dt.float32
BF16 = mybir.dt.bfloat16
BF = ml_dtypes.bfloat16

B, S, H = 2, 2048, 4096
NH, NKV, HD = 32, 8, 128
N_CORES = 8
GROUPS = [[0, 1, 2, 3, 4, 5, 6, 7]]

HL = 4                 # local q heads
OC = H // N_CORES      # 512 local out cols
TC = 512               # token chunk (proj granularity)
NCB = S // TC          # 4 chunks per batch
KT = H // 128          # 32 contraction tiles for the projections
SCALE = float(HD ** -0.5)

LAST_RESULT = None
_BUILT = {}


def _build():
    nc = bacc.Bacc("TRN2", debug=False, num_devices=N_CORES)

    xt_d = nc.dram_tensor("xt", [H, B * S], BF16, kind="ExternalInput").ap()
    cos_d = nc.dram_tensor("cos_t", [HD, B * S], F32, kind="ExternalInput").ap()
    sin_d = nc.dram_tensor("sin_t", [HD, B * S], F32, kind="ExternalInput").ap()
    wq_d = nc.dram_tensor("wq_s", [128, HL * KT * 128], BF16,
                          kind="ExternalInput").ap()
    wk_d = nc.dram_tensor("wk_s", [128, KT * 128], BF16,
                          kind="ExternalInput").ap()
    wv_d = nc.dram_tensor("wv_s", [128, KT * 128], BF16,
                          kind="ExternalInput").ap()
    wo_d = nc.dram_tensor("wo_s", [128, 4 * KT * 128], BF16,
                          kind="ExternalInput").ap()
    mask_d = nc.dram_tensor("masks", [HD, 4 * TC], BF16,
                            kind="ExternalInput").ap()
    ones_d = nc.dram_tensor("onesb", [128, 128], BF16,
                            kind="ExternalInput").ap()
    iden_d = nc.dram_tensor("ident", [128, 128], BF16,
                            kind="ExternalInput").ap()
    out_d = nc.dram_tensor("out_t", [OC, B * S], F32, kind="ExternalOutput").ap()

    with tile.TileContext(nc) as tc:
        with tc.tile_pool(name="sb", bufs=1) as sb, \
             tc.tile_pool(name="ps", bufs=1, space="PSUM") as ps, \
             tc.tile_pool(name="dr", bufs=1, space="DRAM") as dr:

            # ---- persistent tiles ----
            wqp = sb.tile([128, HL * KT * 128], BF16)   # 4MB
            wkp = sb.tile([128, KT * 128], BF16)        # 1MB
            wvp = sb.tile([128, KT * 128], BF16)        # 1MB
            wop = sb.tile([128, 4 * KT * 128], BF16)    # 4MB
            cos_sb = sb.tile([HD, B * S], F32)          # 2MB
            sin_sb = sb.tile([HD, B * S], F32)          # 2MB
            mask_sb = sb.tile([HD, 4 * TC], BF16)
            ones_sb = sb.tile([128, 128], BF16)
            iden_sb = sb.tile([128, 128], BF16)
            ktb = sb.tile([128, B * S], BF16)           # roped K^T, 1MB
            vb = sb.tile([128, (B * S // 128) * 128], BF16)  # V [t, d], 1MB

            QKT = KT * 128 // 4  # quarter-panel cols (1024)

            def load_qkv_quarter(q):
                for h in range(HL):
                    o = h * KT * 128 + q * QKT
                    nc.sync.dma_start(wqp[:, o:o + QKT], wq_d[:, o:o + QKT])
                nc.sync.dma_start(wkp[:, q * QKT:(q + 1) * QKT],
                                  wk_d[:, q * QKT:(q + 1) * QKT])
                nc.sync.dma_start(wvp[:, q * QKT:(q + 1) * QKT],
                                  wv_d[:, q * QKT:(q + 1) * QKT])

            load_qkv_quarter(0)

            def late_loads(kt):
                """remaining startup DMAs, interleaved into chunk-0 X^T DMAs."""
                if kt == 1:
                    load_qkv_quarter(1)
                elif kt == 3:
                    nc.sync.dma_start(cos_sb[:], cos_d[:])
                elif kt == 5:
                    nc.sync.dma_start(sin_sb[:], sin_d[:])
                elif kt == 7:
                    nc.sync.dma_start(mask_sb[:], mask_d[:])
                    nc.sync.dma_start(ones_sb[:], ones_d[:])
                    nc.sync.dma_start(iden_sb[:], iden_d[:])
                elif kt == 9:
                    load_qkv_quarter(2)
                elif kt == 13:
                    load_qkv_quarter(3)
                elif kt in (18, 22, 26, 30):
                    i = (kt - 18) // 4  # wo quarter
                    nc.sync.dma_start(wop[:, i * QKT * 4:(i + 1) * QKT * 4],
                                      wo_d[:, i * QKT * 4:(i + 1) * QKT * 4])

            def rope(dst, pq, t0g):
                """dst (bf16 [128, TC]) = rope of pq (fp32 PSUM [128, TC]).
                Swap DMAs ride the Scalar HWDGE ring, right behind the ACT
                copy that produces their input."""
                qf = sb.tile([128, TC], F32, tag="qf", bufs=2, name="qf")
                nc.scalar.copy(qf[:], pq[:])
                qs = sb.tile([128, TC], F32, tag="qs", bufs=2, name="qs")
                nc.scalar.dma_start(qs[0:64, :], qf[64:128, :])
                nc.scalar.dma_start(qs[64:128, :], qf[0:64, :])
                nc.vector.tensor_tensor(
                    qf[:], qf[:], cos_sb[:, t0g:t0g + TC], mybir.AluOpType.mult)
                nc.vector.tensor_tensor(
                    qs[:], qs[:], sin_sb[:, t0g:t0g + TC], mybir.AluOpType.mult)
                nc.vector.tensor_tensor(dst, qf[:], qs[:], mybir.AluOpType.add)

            def proj(c):
                """kt-outer Q/K/V^T projections + rope for chunk c."""
                b, ci = c // NCB, c % NCB
                t0g = b * S + ci * TC
                pq = [ps.tile([128, TC], F32, tag="quad", bufs=4,
                              name=f"pq{h}") for h in range(HL)]
                pk = ps.tile([128, TC], F32, tag="duo", bufs=2, name="pk")
                pvt = ps.tile([128, TC], F32, tag="duo", bufs=2, name="pvt")
                for kt in range(KT):
                    xt = sb.tile([128, TC], BF16, tag="xt", bufs=10, name="xt")
                    nc.sync.dma_start(
                        xt[:], xt_d[kt * 128:(kt + 1) * 128, t0g:t0g + TC])
                    if c == 0:
                        late_loads(kt)
                    st, sp = (kt == 0), (kt == KT - 1)
                    for h in range(HL):
                        nc.tensor.matmul(
                            pq[h][:],
                            wqp[:, h * KT * 128 + kt * 128:
                                h * KT * 128 + (kt + 1) * 128],
                            xt[:], start=st, stop=sp)
                    nc.tensor.matmul(
                        pk[:], wkp[:, kt * 128:(kt + 1) * 128], xt[:],
                        start=st, stop=sp)
                    nc.tensor.matmul(
                        pvt[:], wvp[:, kt * 128:(kt + 1) * 128], xt[:],
                        start=st, stop=sp)
                # K first (every S^T of this chunk needs it), then V
                # (PE transpose via identity), then Q heads.
                rope(ktb[:, t0g:t0g + TC], pk, t0g)
                vts = sb.tile([128, TC], BF16, tag="vts", bufs=1, name="vts")
                nc.scalar.copy(vts[:], pvt[:])
                ptr = ps.tile([128, TC], BF16, tag="uno", bufs=2, name="ptr")
                for tb in range(TC // 128):
                    nc.tensor.matmul(
                        ptr[:, tb * 128:(tb + 1) * 128],
                        vts[:, tb * 128:(tb + 1) * 128], iden_sb[:],
                        is_transpose=True,
                        start=(tb == 0), stop=(tb == TC // 128 - 1))
                blk0 = t0g // 128
                nc.vector.tensor_copy(
                    vb[:, blk0 * 128:blk0 * 128 + TC], ptr[:])
                qtb = sb.tile([128, HL * TC], BF16, tag="qt", bufs=2,
                              name="qtb")
                for h in range(HL):
                    rope(qtb[:, h * TC:(h + 1) * TC], pq[h], t0g)
                return qtb

            def attn_head(job, h, qtb, attnb):
                c, q0l, tn = job
                b, ci = c // NCB, c % NCB
                kb0 = b * S // 128                      # batch base k-tile
                nkt = (ci * TC + q0l + tn) // 128
                dg0 = (ci * TC + q0l) // 128            # first masked k-tile
                qsl = qtb[:, h * TC + q0l:h * TC + q0l + tn]
                pa = ps.tile([128, tn], F32, tag="duo", bufs=2, name="pa")
                pd = ps.tile([128, tn], F32, tag="uno", bufs=2, name="pd")
                for kt in range(nkt):
                    sps = ps.tile([128, tn], F32, tag="quad", bufs=4,
                                  name="sps")
                    nc.tensor.matmul(
                        sps[:],
                        ktb[:, (kb0 + kt) * 128:(kb0 + kt + 1) * 128],
                        qsl, start=True, stop=True)
                    pt = sb.tile([128, tn], BF16, tag="pt", bufs=3, name="pt")
                    nc.scalar.activation(
                        pt[:], sps[:], mybir.ActivationFunctionType.Exp,
                        scale=SCALE)
                    if kt >= dg0:
                        j = kt - 4 * ci
                        nc.vector.tensor_tensor(
                            pt[:], pt[:],
                            mask_sb[:, j * TC + q0l:j * TC + q0l + tn],
                            mybir.AluOpType.mult)
                    nc.tensor.matmul(
                        pa[:], vb[:, (kb0 + kt) * 128:(kb0 + kt + 1) * 128],
                        pt[:], start=(kt == 0), stop=(kt == nkt - 1))
                    nc.tensor.matmul(
                        pd[:], ones_sb[:], pt[:],
                        start=(kt == 0), stop=(kt == nkt - 1))
                rc = sb.tile([128, tn], F32, tag="rc", bufs=2, name="rc")
                nc.vector.reciprocal_approx_fast(rc[:], pd[:])
                nc.vector.tensor_tensor(
                    attnb[:, h * tn:(h + 1) * tn], pa[:], rc[:],
                    mybir.AluOpType.mult)

            def outproj_block(job, oc, af):
                c, q0l, tn = job
                b, ci = c // NCB, c % NCB
                t0 = b * S + ci * TC + q0l
                po = ps.tile([128, tn], F32, tag="uno", bufs=2, name="po")
                for dt in range(KT):
                    nc.tensor.matmul(
                        po[:],
                        wop[:, oc * KT * 128 + dt * 128:
                            oc * KT * 128 + (dt + 1) * 128],
                        af[:, dt * tn:(dt + 1) * tn],
                        start=(dt == 0), stop=(dt == KT - 1))
                ot = sb.tile([128, tn], F32, tag="ot", bufs=2, name="ot")
                nc.scalar.copy(ot[:], po[:])
                nc.gpsimd.dma_start(
                    out_d[oc * 128:(oc + 1) * 128, t0:t0 + tn], ot[:])

            def allgather(job, attnb):
                c, q0l, tn = job
                cci = dr.tile([HL * 128, tn], BF16, tag="cci", bufs=3,
                              name="cci")
                cco = dr.tile([H, tn], BF16, tag="cco", bufs=3, name="cco")
                nc.gpsimd.dma_start(
                    cci.rearrange("(h p) t -> p h t", p=128),
                    attnb.rearrange("p (h t) -> p h t", h=HL))
                nc.gpsimd.collective_compute(
                    "AllGather", mybir.AluOpType.bypass,
                    replica_groups=GROUPS, ins=[cci[:]], outs=[cco[:]])
                return cco

            def load_af(pending):
                pjob, pcco = pending
                tn = pjob[2]
                af = sb.tile([128, KT * tn], BF16, tag="af", bufs=1,
                             name="af")
                nc.gpsimd.dma_start(
                    af.rearrange("p (d t) -> p d t", d=KT),
                    pcco.rearrange("(d p) t -> p d t", p=128))
                return af

            # jobs: full 512-token chunks 0..6, then chunk 7 in
            # shrinking pieces so the final AllGather tail is small
            LC = B * NCB - 1
            jobs = [(c, 0, TC) for c in range(LC)]
            jobs += [(LC, 0, 256), (LC, 256, 128), (LC, 384, 128)]

            pendings = []  # (job, cco) awaiting output projection
            qtbs = {0: proj(0)}
            for job in jobs:
                c, q0l, tn = job
                if q0l == 0 and c + 1 < B * NCB:
                    qtbs[c + 1] = proj(c + 1)
                qtb = qtbs[c]
                # outproj lags two jobs behind its AllGather, so the af
                # transfer has a full window of slack.
                pending = pendings.pop(0) if len(pendings) >= 2 else None
                af = load_af(pending) if pending is not None else None
                attnb = sb.tile([128, HL * tn], BF16, tag="attn", bufs=1,
                                name="attnb")
                for h in range(HL):
                    attn_head(job, h, qtb, attnb)
                    if pending is not None and h >= 1:
                        outproj_block(pending[0], h - 1, af)
                if pending is not None:
                    outproj_block(pending[0], 3, af)
                cco = allgather(job, attnb)
                pendings.append((job, cco))
            # drain the remaining output projections
            for pending in pendings:
                af = load_af(pending)
                for oc in range(4):
                    outproj_block(pending[0], oc, af)

    nc.compile()
    return nc


def _get_nc():
    if "nc" not in _BUILT:
        _BUILT["nc"] = _build()
    return _BUILT["nc"]


def _panelize(w):
    """[4096, 128] fp32 -> [128, 32*128] bf16 lhsT/rhs panel."""
    return np.ascontiguousarray(
        w.reshape(KT, 128, 128).transpose(1, 0, 2).reshape(128, KT * 128)
    ).astype(BF)


def kernel(hidden_states, cos, sin, wq, wk, wv, wo):
    global LAST_RESULT
    nc = _get_nc()

    hidden_states = np.asarray(hidden_states, dtype=np.float32)
    cos = np.asarray(cos, dtype=np.float32)
    sin = np.asarray(sin, dtype=np.float32)
    wq = np.asarray(wq, dtype=np.float32)
    wk = np.asarray(wk, dtype=np.float32)
    wv = np.asarray(wv, dtype=np.float32)
    wo = np.asarray(wo, dtype=np.float32)

    # host-side shard prep (bf16 casts + panel layouts)
    xt = np.concatenate([hidden_states[0].T, hidden_states[1].T],
                        axis=1).astype(BF)                    # [H, B*S]
    cos_t = np.concatenate([cos[0].T, cos[1].T], axis=1)      # [HD, B*S]
    cos_t = np.ascontiguousarray(cos_t, dtype=np.float32)
    sin_t = np.concatenate([sin[0].T, sin[1].T], axis=1).copy()
    sin_t = np.ascontiguousarray(sin_t, dtype=np.float32)
    sin_t[0:64, :] *= -1.0                                    # rotate_half sign

    masks = np.zeros((HD, 4 * TC), dtype=BF)
    for j in range(4):
        m = (np.arange(HD)[:, None] + 128 * j) <= np.arange(TC)[None, :]
        masks[:, j * TC:(j + 1) * TC] = m.astype(BF)
    onesb = np.ones((128, 128), dtype=BF)
    ident = np.eye(128, dtype=np.float32).astype(BF)

    in_maps = []
    for r in range(N_CORES):
        wq_p = np.concatenate(
            [_panelize(wq[:, (4 * r + h) * 128:(4 * r + h + 1) * 128])
             for h in range(HL)], axis=1)
        wo_p = np.concatenate(
            [_panelize(wo[:, r * OC + oc * 128:r * OC + (oc + 1) * 128])
             for oc in range(4)], axis=1)
        in_maps.append({
            "xt": xt,
            "cos_t": cos_t,
            "sin_t": sin_t,
            "wq_s": wq_p,
            "wk_s": _panelize(wk[:, r * 128:(r + 1) * 128]),
            "wv_s": _panelize(wv[:, r * 128:(r + 1) * 128]),
            "wo_s": wo_p,
            "masks": masks,
            "onesb": onesb,
            "ident": ident,
        })

    res = run_bass_kernel_spmd(nc, in_maps, core_ids=list(range(N_CORES)))
    LAST_RESULT = res

    out = np.empty((B, S, H), dtype=np.float32)
    for r in range(N_CORES):
        o = res.results[r]["out_t"]  # [OC, B*S]
        for b in range(B):
            out[b, :, r * OC:(r + 1) * OC] = o[:, b * S:(b + 1) * S].T
    return out
